# revision 1
# baseline (speedup 1.0000x reference)
"""AttentionBlock (GroupNorm + 1x1-conv QKV self-attention + residual) on 8 TRN2 cores.

Sharding: data-parallel over batch B=4 x sequence-parallel over the 4096
tokens (2 cores per batch element, each handling 2048 query rows; K/V and
GroupNorm are computed redundantly per core pair — they are cheap relative
to attention).

Per-core device kernel (attention matmuls in bf16, GN stats + residual fp32):
  - x is shipped bf16 (matmul/stats operand) + the core's query-half in fp32
    (exact residual); GroupNorm stats overlap the input DMA (bn_stats per
    arriving piece, group-combine via tiny indicator matmuls on the PE).
  - GroupNorm is folded into the QKV weights: h = scale_c*x + shift_c, so
    q/k/v come straight from x with per-channel-scaled weights + effective
    biases; all bias terms (bq/bk/bv/bp + GN shifts) collapse into two
    per-partition vectors applied off the critical path.
  - q/k are computed 2x-replicated across partition strips via col-packed
    (tile_position) projection matmuls, enabling 2x row-packed S^T matmuls
    (K=32 contraction): 2 m-blocks land concurrently in one double-buffered
    2-bank PSUM tile, consumed by a single [128,1024] exp on the scalar
    engine (softmax scale fused into the activation; S range is ~±0.8 so no
    max-subtraction is needed).
  - Softmax denominator: 2x col-packed ones-matmuls accumulate P column sums
    per strip; strips are mask-combined on DVE, partition-all-reduced on the
    (otherwise idle) GpSimd engine, reciprocal on DVE ordered after the pj
    evacuations so it never stalls the PE.
  - P*V accumulates over m-blocks into a 2-bank PSUM tile as out_att[e, n];
    the output projection consumes the bf16 evacuation; each chunk's epilogue
    is software-pipelined into the next chunk's S^T/exp stream.
"""
import sys

sys.path.insert(0, "/opt/trn_rl_repo")

import ml_dtypes
import numpy as np

import concourse.bass as bass
import concourse.bass_isa as bass_isa
import concourse.tile as tile
from concourse.tile_rust import add_dep_helper
from concourse import bacc, mybir
from concourse.bass_utils import run_bass_kernel_spmd

F32 = mybir.dt.float32
BF16 = mybir.dt.bfloat16

B, C, H, W = 4, 256, 64, 64
N = H * W          # 4096 tokens
NQ = N // 2        # 2048 query rows per core
D = C // 8         # 32 qk dim
G = 32             # groups
GS = C // G        # 8 channels per group
EPS = 1e-5
P = 128            # partitions
CT = C // P        # 2 channel tiles
CH = 512           # nq chunk
NCH = NQ // CH     # 4 chunks
MB = 128           # m block
NMB = N // MB      # 32 m blocks
NG = NMB // 4      # 8 groups of 4 m-blocks per chunk
SM_SCALE = float(D) ** -0.5

_CACHE = {}
_last_in_maps = None


def _build():
    if "nc" in _CACHE:
        return _CACHE["nc"]

    nc = bacc.Bacc("TRN2", target_bir_lowering=False, debug=False, num_devices=8)

    x_ext = nc.declare_dram_parameter("x", [C, N], BF16, isOutput=False)
    xq_ext = nc.declare_dram_parameter("xq", [C, NQ], F32, isOutput=False)
    wqt_ext = nc.declare_dram_parameter("wqt", [C, D], F32, isOutput=False)
    wkt_ext = nc.declare_dram_parameter("wkt", [C, D], F32, isOutput=False)
    wvt_ext = nc.declare_dram_parameter("wvt", [C, C], F32, isOutput=False)
    wpt_ext = nc.declare_dram_parameter("wpt", [C, C], F32, isOutput=False)
    bq_ext = nc.declare_dram_parameter("bq", [D, 1], F32, isOutput=False)
    bk_ext = nc.declare_dram_parameter("bk", [D, 1], F32, isOutput=False)
    bv_ext = nc.declare_dram_parameter("bv", [C, 1], F32, isOutput=False)
    bp_ext = nc.declare_dram_parameter("bp", [C, 1], F32, isOutput=False)
    gamma_ext = nc.declare_dram_parameter("gamma", [C, 1], F32, isOutput=False)
    beta_ext = nc.declare_dram_parameter("beta", [C, 1], F32, isOutput=False)
    ind16_ext = nc.declare_dram_parameter("ind16", [P, G // CT], F32, isOutput=False)
    indb_ext = nc.declare_dram_parameter("indb", [G // CT, P], F32, isOutput=False)
    out_ext = nc.declare_dram_parameter("out", [C, NQ], F32, isOutput=True)

    GT = G // CT  # 16 groups per channel tile
    XP = N // 2   # x DMA piece size (overlap DMA with stats)

    with tile.TileContext(nc) as tc:
        with tc.tile_pool(name="const", bufs=1) as const, \
             tc.tile_pool(name="small", bufs=1) as small:
            # input DMAs issued first: the x descriptors must not queue
            # behind the small weight/bias transfers on the same engines
            x_r = [const.tile([P, N], BF16, tag=f"xr{t}", name=f"xr{t}") for t in range(CT)]
            for t in range(CT):
                cs = slice(t * P, (t + 1) * P)
                for pc in range(N // XP):
                    ps_ = slice(pc * XP, (pc + 1) * XP)
                    qeng = [nc.sync, nc.gpsimd, nc.scalar, nc.sync][(t * (N // XP) + pc) % 4]
                    qeng.dma_start(out=x_r[t][:, ps_], in_=x_ext[cs, ps_])

            # ---- persistent tiles ----
            wqt_sb, wkt_sb, wvt_sb, wpt_sb = [], [], [], []
            gamma_sb, beta_sb, bv_sb, bp_sb = [], [], [], []
            for t in range(CT):
                cs = slice(t * P, (t + 1) * P)
                w1 = const.tile([P, D], F32, tag=f"wqt{t}", name=f"wqt{t}")
                nc.gpsimd.dma_start(out=w1, in_=wqt_ext[cs, :])
                wqt_sb.append(w1)
                w2 = const.tile([P, D], F32, tag=f"wkt{t}", name=f"wkt{t}")
                nc.gpsimd.dma_start(out=w2, in_=wkt_ext[cs, :])
                wkt_sb.append(w2)
                w3 = const.tile([P, C], F32, tag=f"wvt{t}", name=f"wvt{t}")
                nc.gpsimd.dma_start(out=w3, in_=wvt_ext[cs, :])
                wvt_sb.append(w3)
                w4 = const.tile([P, C], F32, tag=f"wpt{t}", name=f"wpt{t}")
                nc.gpsimd.dma_start(out=w4, in_=wpt_ext[cs, :])
                wpt_sb.append(w4)
                for lst, ext, nm in (
                    (gamma_sb, gamma_ext, "gam"),
                    (beta_sb, beta_ext, "bet"),
                    (bv_sb, bv_ext, "bv"),
                    (bp_sb, bp_ext, "bp"),
                ):
                    tl = small.tile([P, 1], F32, tag=f"{nm}{t}", name=f"{nm}{t}")
                    nc.sync.dma_start(out=tl, in_=ext[cs, :])
                    lst.append(tl)
            bq_sb = small.tile([D, 1], F32, tag="bq")
            nc.sync.dma_start(out=bq_sb, in_=bq_ext[:])
            bk_sb = small.tile([D, 1], F32, tag="bk")
            nc.sync.dma_start(out=bk_sb, in_=bk_ext[:])
            ind16_sb = small.tile([P, GT], F32, tag="ind16")
            nc.sync.dma_start(out=ind16_sb, in_=ind16_ext[:])
            indb_sb = small.tile([GT, P], F32, tag="indb")
            nc.sync.dma_start(out=indb_sb, in_=indb_ext[:])
            onec_h = small.tile([P, 1], BF16, tag="onech")
            nc.vector.memset(onec_h, 1.0)
            mask4_sb = small.tile([P, 1], F32, tag="mask4")
            nc.vector.memset(mask4_sb, 0.0)
            nc.vector.memset(mask4_sb[0:1, :], 1.0)
            nc.vector.memset(mask4_sb[32:33, :], 1.0)
            eps_sb = small.tile([GT, 1], F32, tag="eps")
            nc.vector.memset(eps_sb, EPS)

            xq_r = [const.tile([P, NQ], BF16, tag=f"xqr{t}", name=f"xqr{t}") for t in range(CT)]
            xqb = [const.tile([P, NQ], F32, tag=f"xqb{t}", name=f"xqb{t}") for t in range(CT)]
            scale_sb = [small.tile([P, 1], F32, tag=f"scale{t}", name=f"scale{t}") for t in range(CT)]
            shift_sb = [small.tile([P, 1], F32, tag=f"shift{t}", name=f"shift{t}") for t in range(CT)]

            # ---- load x; GroupNorm stats overlapped with DMA ----
            with tc.tile_pool(name="ld", bufs=2) as ld, \
                 tc.tile_pool(name="gn", bufs=2) as gn, \
                 tc.tile_pool(name="gnps", bufs=1, space="PSUM") as gnps:
                xq_f = []
                for t in range(CT):
                    cs = slice(t * P, (t + 1) * P)
                    stats = gn.tile([P, 8, nc.vector.BN_STATS_DIM], F32, tag="st")
                    for pc in range(N // XP):
                        for s in range(XP // 512):
                            si = pc * (XP // 512) + s
                            nc.vector.bn_stats(
                                out=stats[:, si, :],
                                in_=x_r[t][:, pc * XP + s * 512: pc * XP + (s + 1) * 512],
                            )
                    xqt = ld.tile([P, NQ], F32, tag=f"xqt{t}", name=f"xqt{t}")
                    (nc.scalar if t else nc.gpsimd).dma_start(out=xqt, in_=xq_ext[cs, :])
                    nc.scalar.activation(
                        out=xq_r[t], in_=xqt,
                        func=mybir.ActivationFunctionType.Copy,
                    )
                    xq_f.append(xqt)

                    mv = gn.tile([P, nc.vector.BN_AGGR_DIM], F32, tag="mv")
                    nc.vector.bn_aggr(out=mv, in_=stats)
                    mx = gn.tile([P, 2], F32, tag="mx")
                    nc.vector.tensor_copy(out=mx[:, 0:1], in_=mv[:, 0:1])
                    msq = gn.tile([P, 1], F32, tag="msq")
                    nc.vector.tensor_mul(out=msq, in0=mv[:, 0:1], in1=mv[:, 0:1])
                    nc.vector.tensor_add(out=mx[:, 1:2], in0=mv[:, 1:2], in1=msq)

                    gps = gnps.tile([GT, 2], F32, tag="gps")
                    nc.tensor.matmul(gps, ind16_sb, mx, start=True, stop=True)
                    gsb = gn.tile([GT, 2], F32, tag="gsb")
                    nc.vector.tensor_copy(out=gsb, in_=gps)
                    mg2 = gn.tile([GT, 1], F32, tag="mg2")
                    nc.vector.tensor_mul(out=mg2, in0=gsb[:, 0:1], in1=gsb[:, 0:1])
                    varg = gn.tile([GT, 1], F32, tag="varg")
                    nc.vector.tensor_sub(out=varg, in0=gsb[:, 1:2], in1=mg2)
                    sd = gn.tile([GT, 1], F32, tag="sd")
                    nc.scalar.activation(
                        out=sd, in_=varg,
                        func=mybir.ActivationFunctionType.Sqrt,
                        bias=eps_sb, scale=1.0,
                    )
                    g2 = gn.tile([GT, 2], F32, tag="g2")
                    nc.vector.tensor_copy(out=g2[:, 0:1], in_=gsb[:, 0:1])
                    nc.vector.reciprocal(out=g2[:, 1:2], in_=sd)

                    bc = gnps.tile([P, 2], F32, tag="bc")
                    nc.tensor.matmul(bc, indb_sb, g2, start=True, stop=True)
                    nc.vector.tensor_mul(out=scale_sb[t], in0=gamma_sb[t], in1=bc[:, 1:2])
                    sh1 = gn.tile([P, 1], F32, tag="sh1")
                    nc.vector.tensor_mul(out=sh1, in0=bc[:, 0:1], in1=scale_sb[t])
                    nc.vector.tensor_sub(out=shift_sb[t], in0=beta_sb[t], in1=sh1)

                # ---- scaled weights + effective biases ----
                wqt_h = [const.tile([P, D], BF16, tag=f"wqth{t}", name=f"wqth{t}") for t in range(CT)]
                wkt_h = [const.tile([P, D], BF16, tag=f"wkth{t}", name=f"wkth{t}") for t in range(CT)]
                wvt_h = [const.tile([P, C], BF16, tag=f"wvth{t}", name=f"wvth{t}") for t in range(CT)]
                wpt_h = [const.tile([P, C], BF16, tag=f"wpth{t}", name=f"wpth{t}") for t in range(CT)]
                for t in range(CT):
                    nc.vector.tensor_scalar_mul(out=wqt_h[t], in0=wqt_sb[t], scalar1=scale_sb[t])
                    nc.vector.tensor_scalar_mul(out=wkt_h[t], in0=wkt_sb[t], scalar1=scale_sb[t])
                    nc.vector.tensor_scalar_mul(out=wvt_h[t], in0=wvt_sb[t], scalar1=scale_sb[t])
                    nc.vector.tensor_copy(out=wpt_h[t], in_=wpt_sb[t])

                with tc.tile_pool(name="bps", bufs=1, space="PSUM") as bps:
                    bq_eff = small.tile([D, 1], F32, tag="bqe")
                    bk_eff = small.tile([D, 1], F32, tag="bke")
                    psq = bps.tile([D, 1], F32, tag="pq")
                    psk = bps.tile([D, 1], F32, tag="pk")
                    for t in range(CT):
                        nc.tensor.matmul(psq, wqt_sb[t], shift_sb[t], start=(t == 0), stop=(t == CT - 1))
                        nc.tensor.matmul(psk, wkt_sb[t], shift_sb[t], start=(t == 0), stop=(t == CT - 1))
                    nc.vector.tensor_add(out=bq_eff, in0=psq, in1=bq_sb)
                    nc.vector.tensor_add(out=bk_eff, in0=psk, in1=bk_sb)
                    # replicate biases across the 2 partition strips
                    bq_rep = small.tile([64, 1], F32, tag="bqrep")
                    bk_rep = small.tile([64, 1], F32, tag="bkrep")
                    for j in range(2):
                        nc.vector.tensor_copy(out=bq_rep[32 * j:32 * (j + 1), :], in_=bq_eff)
                        nc.vector.tensor_copy(out=bk_rep[32 * j:32 * (j + 1), :], in_=bk_eff)

                    bv_eff = [small.tile([P, 1], F32, tag=f"bve{e}", name=f"bve{e}") for e in range(CT)]
                    for e in range(CT):
                        ps3 = bps.tile([P, 1], F32, tag=f"pv{e}", name=f"psv{e}")
                        for t in range(CT):
                            nc.tensor.matmul(
                                ps3, wvt_sb[t][:, e * P:(e + 1) * P], shift_sb[t],
                                start=(t == 0), stop=(t == CT - 1),
                            )
                        nc.vector.tensor_add(out=bv_eff[e], in0=ps3, in1=bv_sb[e])
                    for f in range(CT):
                        ps4 = bps.tile([P, 1], F32, tag=f"pp{f}", name=f"psp{f}")
                        for e in range(CT):
                            nc.tensor.matmul(
                                ps4, wpt_sb[e][:, f * P:(f + 1) * P], bv_eff[e],
                                start=(e == 0), stop=(e == CT - 1),
                            )
                        bp_eff = small.tile([P, 1], F32, tag=f"bpe{f}", name=f"bpe{f}")
                        nc.vector.tensor_add(out=bp_eff, in0=ps4, in1=bp_sb[f])
                        nc.vector.tensor_scalar_add(out=xqb[f], in0=xq_f[f], scalar1=bp_eff)

            # ---- q/k (4x partition-replicated via col-packed matmuls) + v^T ----
            q_rep = const.tile([64, NQ], BF16, tag="qrep")
            k_rep = const.tile([64, N], BF16, tag="krep")
            vt_h = const.tile([P, NMB, C], BF16, tag="vth")
            with tc.tile_pool(name="qkps", bufs=1, space="PSUM") as qkps, \
                 tc.tile_pool(name="vtps", bufs=2, space="PSUM") as vtps:
                for ch2 in range(NQ // (2 * CH)):
                    qp = qkps.tile([64, 2 * CH], F32, tag="qkp", bufs=2, name="qp")
                    for half in range(2):
                        ns = slice((2 * ch2 + half) * CH, (2 * ch2 + half + 1) * CH)
                        hs = slice(half * CH, (half + 1) * CH)
                        for t in range(CT):
                            for j in range(2):
                                nc.tensor.matmul(
                                    qp[32 * j:32 * (j + 1), hs], wqt_h[t], xq_r[t][:, ns],
                                    start=(t == 0), stop=(t == CT - 1),
                                    tile_position=(0, 32 * j),
                                )
                    ns2 = slice(2 * ch2 * CH, 2 * (ch2 + 1) * CH)
                    nc.scalar.activation(
                        out=q_rep[:, ns2], in_=qp,
                        func=mybir.ActivationFunctionType.Identity,
                        bias=bq_rep[0:64, :], scale=1.0,
                    )
                for ch2 in range(N // (2 * CH)):
                    kp = qkps.tile([64, 2 * CH], F32, tag="qkp", bufs=2, name="kp")
                    for half in range(2):
                        ns = slice((2 * ch2 + half) * CH, (2 * ch2 + half + 1) * CH)
                        hs = slice(half * CH, (half + 1) * CH)
                        for t in range(CT):
                            for j in range(2):
                                nc.tensor.matmul(
                                    kp[32 * j:32 * (j + 1), hs], wkt_h[t], x_r[t][:, ns],
                                    start=(t == 0), stop=(t == CT - 1),
                                    tile_position=(0, 32 * j),
                                )
                    ns2 = slice(2 * ch2 * CH, 2 * (ch2 + 1) * CH)
                    nc.scalar.activation(
                        out=k_rep[:, ns2], in_=kp,
                        func=mybir.ActivationFunctionType.Identity,
                        bias=bk_rep[0:64, :], scale=1.0,
                    )
                # v^T in 4-m-block granules: [128, 1024] 2-bank psum, one wide copy
                for vg in range(NMB // 4):
                    vp = vtps.tile([P, 4, C], F32, tag="vp")
                    for mloc in range(4):
                        mb = vg * 4 + mloc
                        ms = slice(mb * MB, (mb + 1) * MB)
                        for t in range(CT):
                            nc.tensor.matmul(
                                vp[:, mloc, :], x_r[t][:, ms], wvt_h[t],
                                start=(t == 0), stop=(t == CT - 1),
                            )
                    nc.vector.tensor_copy(out=vt_h[:, vg * 4:(vg + 1) * 4, :], in_=vp)

            # ---- attention ----
            with tc.tile_pool(name="stps", bufs=2, space="PSUM") as stps, \
                 tc.tile_pool(name="attps", bufs=1, space="PSUM") as attps, \
                 tc.tile_pool(name="rsps", bufs=1, space="PSUM") as rsps, \
                 tc.tile_pool(name="pp", bufs=6) as pp, \
                 tc.tile_pool(name="attsb", bufs=4) as attsb, \
                 tc.tile_pool(name="osb", bufs=4) as osb, \
                 tc.tile_pool(name="rsb", bufs=2) as rsb:
                NG2 = NMB // 2
                pend_a = None  # epilogue part A payload of the previous chunk
                pend_b = None  # epilogue part B payload

                def emit_epilogue(ep, final=False):
                    ns_p, att2_p, rs_p = ep
                    rec_bc = rsb.tile([P, CH], F32, tag="recbc")
                    if final:
                        # tail chunk: the whole denominator chain (mask, gpsimd
                        # reduce, reciprocal) is issued first so it overlaps the
                        # att evacuation + projection instead of serializing
                        rs_sb = rsb.tile([P, CH], F32, tag="rssb")
                        nc.vector.tensor_scalar_mul(out=rs_sb, in0=rs_p, scalar1=mask4_sb)
                        rsum = rsb.tile([P, CH], F32, tag="rsum")
                        nc.gpsimd.partition_all_reduce(
                            rsum, rs_sb, channels=P, reduce_op=bass_isa.ReduceOp.add,
                        )
                        nc.vector.reciprocal(out=rec_bc, in_=rsum)
                    att_sb2 = attsb.tile([P, CT * CH], BF16, tag="attsb2")
                    nc.vector.tensor_copy(out=att_sb2, in_=att2_p)
                    if not final:
                        rs_sb = rsb.tile([P, CH], F32, tag="rssb")
                        nc.vector.tensor_scalar_mul(out=rs_sb, in0=rs_p, scalar1=mask4_sb)
                        rsum = rsb.tile([P, CH], F32, tag="rsum")
                        nc.gpsimd.partition_all_reduce(
                            rsum, rs_sb, channels=P, reduce_op=bass_isa.ReduceOp.add,
                        )
                    pjs = []
                    pjc_inst = None
                    for f in range(CT):
                        pj = rsps.tile([P, CH], F32, tag="pj", name=f"pj{f}")
                        for e in range(CT):
                            nc.tensor.matmul(
                                pj, wpt_h[e][:, f * P:(f + 1) * P],
                                att_sb2[:, e * CH:(e + 1) * CH],
                                start=(e == 0), stop=(e == CT - 1),
                            )
                        # plain copy releases the pj bank without waiting on
                        # the denominator
                        pjc = osb.tile([P, CH], F32, tag="pjc", name=f"pjc{f}")
                        pjc_inst = nc.vector.tensor_copy(out=pjc, in_=pj)
                        pjs.append(pjc)
                    if not final:
                        rec_inst = nc.vector.reciprocal(out=rec_bc, in_=rsum)
                        # keep the reciprocal behind the pj copies in the DVE
                        # stream so it never blocks the att cast -> proj path
                        add_dep_helper(rec_inst.ins, pjc_inst.ins, sync=False,
                                       reason="recip after pj copies")
                    for f in range(CT):
                        fs = slice(f * P, (f + 1) * P)
                        t1 = osb.tile([P, CH], F32, tag="t1")
                        nc.vector.tensor_mul(out=t1, in0=pjs[f], in1=rec_bc)
                        o = osb.tile([P, CH], F32, tag="o")
                        nc.vector.tensor_add(out=o, in0=t1, in1=xqb[f][:, ns_p])
                        nc.sync.dma_start(out=out_ext[fs, ns_p], in_=o)

                for ch in range(NCH):
                    ns = slice(ch * CH, (ch + 1) * CH)
                    att2 = attps.tile([P, CT * CH], F32, tag="att2")
                    rs = rsps.tile([P, CH], F32, tag="rs")

                    p_tiles = [None] * NG2
                    for g in range(NG2 + 1):
                        if g < NG2:
                            # 2 row-packed S^T matmuls, issued two groups ahead
                            # of their consumers so the exp stream never waits
                            stg = stps.tile([P, 2 * CH], F32, tag="stg")
                            for j in range(2):
                                mb = g * 2 + j
                                nc.tensor.matmul(
                                    stg[:, j * CH:(j + 1) * CH],
                                    k_rep[32 * j:32 * (j + 1), mb * MB:(mb + 1) * MB],
                                    q_rep[32 * j:32 * (j + 1), ns],
                                    start=True, stop=True,
                                    tile_position=(32 * j, 0),
                                )
                            pg = pp.tile([P, 2 * CH], BF16, tag="pg")
                            nc.scalar.activation(
                                out=pg, in_=stg,
                                func=mybir.ActivationFunctionType.Exp,
                                scale=SM_SCALE,
                            )
                            p_tiles[g] = pg
                        if g == 1 and pend_a is not None:
                            emit_epilogue(pend_a)
                            pend_a = None
                        if g >= 1:
                            gp = g - 1
                            pg = p_tiles[gp]
                            for j in range(2):
                                nc.tensor.matmul(
                                    rs[32 * j:32 * j + 1, :],
                                    onec_h, pg[:, j * CH:(j + 1) * CH],
                                    start=(gp == 0), stop=(gp == NG2 - 1),
                                    tile_position=(0, 32 * j),
                                )
                            for j in range(2):
                                mb = gp * 2 + j
                                for e in range(CT):
                                    nc.tensor.matmul(
                                        att2[:, e * CH:(e + 1) * CH],
                                        vt_h[:, mb, e * P:(e + 1) * P],
                                        pg[:, j * CH:(j + 1) * CH],
                                        start=(mb == 0), stop=(mb == NMB - 1),
                                    )
                    pend_a = (ns, att2, rs)
                emit_epilogue(pend_a, final=True)

    nc.compile()
    _CACHE["nc"] = nc
    return nc


def kernel(x, gamma, beta, wq, bq, wk, bk, wv, bv, wp, bp):
    x = np.ascontiguousarray(np.asarray(x, dtype=np.float32))
    nc = _build()

    GT = G // CT
    ind16 = np.zeros((P, GT), np.float32)
    for c in range(P):
        ind16[c, c // GS] = 1.0 / GS
    indb = np.zeros((GT, P), np.float32)
    for c in range(P):
        indb[c // GS, c] = 1.0

    common = {
        "wqt": np.ascontiguousarray(np.asarray(wq, np.float32).T),
        "wkt": np.ascontiguousarray(np.asarray(wk, np.float32).T),
        "wvt": np.ascontiguousarray(np.asarray(wv, np.float32).T),
        "wpt": np.ascontiguousarray(np.asarray(wp, np.float32).T),
        "bq": np.asarray(bq, np.float32).reshape(D, 1),
        "bk": np.asarray(bk, np.float32).reshape(D, 1),
        "bv": np.asarray(bv, np.float32).reshape(C, 1),
        "bp": np.asarray(bp, np.float32).reshape(C, 1),
        "gamma": np.asarray(gamma, np.float32).reshape(C, 1),
        "beta": np.asarray(beta, np.float32).reshape(C, 1),
        "ind16": ind16,
        "indb": indb,
    }

    xf = x.reshape(B, C, N)
    xh = np.ascontiguousarray(xf.astype(ml_dtypes.bfloat16))
    in_maps = []
    for core in range(8):
        b, half = core // 2, core % 2
        m = dict(common)
        m["x"] = xh[b]
        m["xq"] = np.ascontiguousarray(xf[b][:, half * NQ:(half + 1) * NQ])
        in_maps.append(m)

    global _last_in_maps
    _last_in_maps = in_maps
    res = run_bass_kernel_spmd(nc, in_maps, list(range(8)))

    y = np.empty((B, C, N), np.float32)
    for core in range(8):
        b, half = core // 2, core % 2
        y[b][:, half * NQ:(half + 1) * NQ] = res.results[core]["out"]
    return y.reshape(B, C, H, W)



# revision 4
# speedup vs baseline: 1.0096x; 1.0096x over previous
"""AttentionBlock (GroupNorm + 1x1-conv QKV self-attention + residual) on 8 TRN2 cores.

Sharding: data-parallel over batch B=4 x sequence-parallel over the 4096
tokens (2 cores per batch element, each handling 2048 query rows; K/V and
GroupNorm are computed redundantly per core pair — they are cheap relative
to attention).

Per-core device kernel — all heavy matmuls in fp8e4 DoubleRow perf mode
(contraction-256 ops run in a single PE pass):
  - 32 dummy matmuls at kernel start keep the PE clock ramped while the
    input DMA + GroupNorm stats run, so real matmuls start at full p-state.
  - x ships as fp8 [128, 2, N] (the 2 = channel k-tiles) + the core's query
    half in fp32 (exact residual). GroupNorm stats are split across engines:
    channel tile 0 via DVE bn_stats, tile 1 via scalar-engine Copy/Square
    activations with accum_out. GN is folded into the QKV weights, which
    are scaled by 64 into fp8's healthy range.
  - q/k/v projections are single DoubleRow matmuls (contraction C=256 via
    k-tiles). v^T pairs 2..15 are produced inside chunk 0 of the attention
    loop (the projection PSUM bank is idle there), so the softmax pipeline
    starts ~10us earlier.
  - S^T uses DoubleRow with a stride-0 broadcast k-tile dim (both tiles
    read the same q/k memory, so S = 2*q'k'; the 1/2 folds into the exp
    scale). exp on the scalar engine, output fp8.
  - The softmax denominator is an all-ones [128,2,128] fp8 DoubleRow
    matmul accumulated over key pairs — its output is already broadcast
    across all 128 partitions: no gpsimd reduce / mask / replication.
    Reciprocal via the single-op reciprocal_approx_fast.
  - P*V accumulates key-block pairs per DoubleRow matmul into att2; the
    epilogue divides by the denominator during the fp8 evacuation
    (att8 = att2 * rec = 64*att), projects with fp8 DoubleRow, and fuses
    residual-add + 1/4096 rescale in one scalar_tensor_tensor.
"""
import sys

sys.path.insert(0, "/opt/trn_rl_repo")

import ml_dtypes
import numpy as np

import concourse.bass as bass
import concourse.tile as tile
from concourse import bacc, mybir
from concourse.bass_utils import run_bass_kernel_spmd

F32 = mybir.dt.float32
BF16 = mybir.dt.bfloat16
FP8 = mybir.dt.float8e4
DR = mybir.MatmulPerfMode.DoubleRow
MULT = mybir.AluOpType.mult
ADD = mybir.AluOpType.add

B, C, H, W = 4, 256, 64, 64
N = H * W          # 4096 tokens
NQ = N // 2        # 2048 query rows per core
D = C // 8         # 32 qk dim
G = 32             # groups
GS = C // G        # 8 channels per group
EPS = 1e-5
P = 128            # partitions
CT = C // P        # 2 channel tiles
CH = 512           # nq chunk
NCH = NQ // CH     # 4 chunks
MB = 128           # m block
NMB = N // MB      # 32 m blocks
NPR = NMB // 2     # 16 m-block pairs
WS = 64.0          # fp8 weight scale
SM_SCALE = float(D) ** -0.5
EXP_SCALE = SM_SCALE / (2.0 * WS * WS)
NWARM = 32         # PE p-state warmup matmuls

_CACHE = {}
_last_in_maps = None


def _build():
    if "nc" in _CACHE:
        return _CACHE["nc"]

    nc = bacc.Bacc("TRN2", target_bir_lowering=False, debug=False, num_devices=8)

    WALL = 2 * D + 2 * C  # wq|wk|wv|wp columns, transposed, concatenated
    x8_ext = nc.declare_dram_parameter("x8", [P, CT * N], FP8, isOutput=False)
    xq_ext = nc.declare_dram_parameter("xq", [C, NQ], F32, isOutput=False)
    wall_ext = nc.declare_dram_parameter("wall", [C, WALL], F32, isOutput=False)
    bc4_ext = nc.declare_dram_parameter("bc4", [C, 4], F32, isOutput=False)  # gamma|beta|bv|bp
    bqk_ext = nc.declare_dram_parameter("bqk", [D, 2], F32, isOutput=False)  # bq|bk
    ind16_ext = nc.declare_dram_parameter("ind16", [P, G // CT], F32, isOutput=False)
    indb_ext = nc.declare_dram_parameter("indb", [G // CT, P], F32, isOutput=False)
    out_ext = nc.declare_dram_parameter("out", [C, NQ], F32, isOutput=True)

    GT = G // CT  # 16 groups per channel tile
    NHALF = N // 2

    with tile.TileContext(nc) as tc:
        with tc.tile_pool(name="const", bufs=1) as const, \
             tc.tile_pool(name="small", bufs=1) as small:
            # ---- PE warmup: keep the clock ramped during the DMA wait ----
            wsrc = small.tile([P, CH], FP8, tag="wsrc")
            nc.vector.memset(wsrc, 1.0)
            ones8 = small.tile([P, 1], FP8, tag="ones8")
            nc.vector.memset(ones8, 1.0)
            with tc.tile_pool(name="warm", bufs=1, space="PSUM") as warm:
                for wi in range(NWARM):
                    wp_ = warm.tile([1, CH], F32, tag="warm", name=f"warm{wi}")
                    nc.tensor.matmul(wp_, ones8, wsrc, start=True, stop=True)

            # ---- input DMAs: x8 quadrants first, then weights, then xq ----
            x8 = const.tile([P, CT, N], FP8, tag="x8", name="x8")
            nc.sync.dma_start(out=x8[:, 0, 0:NHALF], in_=x8_ext[:, 0:NHALF])
            nc.scalar.dma_start(out=x8[:, 0, NHALF:N], in_=x8_ext[:, NHALF:N])
            nc.gpsimd.dma_start(out=x8[:, 1, 0:NHALF], in_=x8_ext[:, N:N + NHALF])
            nc.sync.dma_start(out=x8[:, 1, NHALF:N], in_=x8_ext[:, N + NHALF:2 * N])

            wall_sb = []
            for t in range(CT):
                cs = slice(t * P, (t + 1) * P)
                wl = const.tile([P, WALL], F32, tag=f"wall{t}", name=f"wall{t}")
                (nc.gpsimd if t == 0 else nc.scalar).dma_start(out=wl, in_=wall_ext[cs, :])
                wall_sb.append(wl)
            wqt_sb = [wall_sb[t][:, 0:D] for t in range(CT)]
            wkt_sb = [wall_sb[t][:, D:2 * D] for t in range(CT)]
            wvt_sb = [wall_sb[t][:, 2 * D:2 * D + C] for t in range(CT)]
            wpt_sb = [wall_sb[t][:, 2 * D + C:WALL] for t in range(CT)]

            bc4_sb = []
            for t in range(CT):
                cs = slice(t * P, (t + 1) * P)
                b4 = small.tile([P, 4], F32, tag=f"bc4{t}", name=f"bc4{t}")
                nc.sync.dma_start(out=b4, in_=bc4_ext[cs, :])
                bc4_sb.append(b4)
            gamma_sb = [bc4_sb[t][:, 0:1] for t in range(CT)]
            beta_sb = [bc4_sb[t][:, 1:2] for t in range(CT)]
            bv_sb = [bc4_sb[t][:, 2:3] for t in range(CT)]
            bp_sb = [bc4_sb[t][:, 3:4] for t in range(CT)]
            bqk_sb = small.tile([D, 2], F32, tag="bqk")
            nc.sync.dma_start(out=bqk_sb, in_=bqk_ext[:])
            bq_sb, bk_sb = bqk_sb[:, 0:1], bqk_sb[:, 1:2]
            ind16_sb = small.tile([P, GT], F32, tag="ind16")
            nc.sync.dma_start(out=ind16_sb, in_=ind16_ext[:])
            indb_sb = small.tile([GT, P], F32, tag="indb")
            nc.sync.dma_start(out=indb_sb, in_=indb_ext[:])
            eps_sb = small.tile([GT, 1], F32, tag="eps")
            nc.vector.memset(eps_sb, EPS)

            xq_f = []
            for t in range(CT):
                cs = slice(t * P, (t + 1) * P)
                xqt = const.tile([P, NQ], F32, tag=f"xqt{t}", name=f"xqt{t}")
                (nc.gpsimd if t == 0 else nc.scalar).dma_start(out=xqt, in_=xq_ext[cs, :])
                xq_f.append(xqt)

            xqb = [const.tile([P, NQ], F32, tag=f"xqb{t}", name=f"xqb{t}") for t in range(CT)]
            scale_sb = [small.tile([P, 1], F32, tag=f"scale{t}", name=f"scale{t}") for t in range(CT)]
            shift_sb = [small.tile([P, 1], F32, tag=f"shift{t}", name=f"shift{t}") for t in range(CT)]

            # ---- GroupNorm stats: tile 0 on DVE (bn_stats), tile 1 on the
            # scalar engine (Copy/Square + accum_out), in DMA-half granules ----
            with tc.tile_pool(name="gn", bufs=2) as gn, \
                 tc.tile_pool(name="gnps", bufs=1, space="PSUM") as gnps:
                mx_t = []
                # tile 0: DVE
                stats = gn.tile([P, 8, nc.vector.BN_STATS_DIM], F32, tag="st")
                for s in range(N // 512):
                    nc.vector.bn_stats(out=stats[:, s, :], in_=x8[:, 0, s * 512:(s + 1) * 512])
                mv = gn.tile([P, nc.vector.BN_AGGR_DIM], F32, tag="mv")
                nc.vector.bn_aggr(out=mv, in_=stats)
                mx0 = gn.tile([P, 2], F32, tag="mx0")
                nc.vector.tensor_copy(out=mx0[:, 0:1], in_=mv[:, 0:1])
                msq = gn.tile([P, 1], F32, tag="msq")
                nc.vector.tensor_mul(out=msq, in0=mv[:, 0:1], in1=mv[:, 0:1])
                nc.vector.tensor_add(out=mx0[:, 1:2], in0=mv[:, 1:2], in1=msq)
                mx_t.append(mx0)
                # tile 1: scalar engine accumulators
                scr = gn.tile([P, NHALF], FP8, tag="scr")
                acc = gn.tile([P, 4], F32, tag="acc")  # sum_h0|sum_h1|sq_h0|sq_h1
                for hh in range(2):
                    xs = x8[:, 1, hh * NHALF:(hh + 1) * NHALF]
                    nc.scalar.activation(
                        out=scr, in_=xs, func=mybir.ActivationFunctionType.Copy,
                        accum_out=acc[:, hh:hh + 1])
                for hh in range(2):
                    xs = x8[:, 1, hh * NHALF:(hh + 1) * NHALF]
                    nc.scalar.activation(
                        out=scr, in_=xs, func=mybir.ActivationFunctionType.Square,
                        accum_out=acc[:, 2 + hh:3 + hh])
                mx1 = gn.tile([P, 2], F32, tag="mx1")
                ssum = gn.tile([P, 2], F32, tag="ssum")
                nc.vector.tensor_add(out=ssum[:, 0:1], in0=acc[:, 0:1], in1=acc[:, 1:2])
                nc.vector.tensor_add(out=ssum[:, 1:2], in0=acc[:, 2:3], in1=acc[:, 3:4])
                nc.vector.tensor_scalar_mul(out=mx1, in0=ssum, scalar1=1.0 / float(N))
                mx_t.append(mx1)

                for t in range(CT):
                    gps = gnps.tile([GT, 2], F32, tag="gps")
                    nc.tensor.matmul(gps, ind16_sb, mx_t[t], start=True, stop=True)
                    gsb = gn.tile([GT, 2], F32, tag="gsb")
                    nc.vector.tensor_copy(out=gsb, in_=gps)
                    mg2 = gn.tile([GT, 1], F32, tag="mg2")
                    nc.vector.tensor_mul(out=mg2, in0=gsb[:, 0:1], in1=gsb[:, 0:1])
                    varg = gn.tile([GT, 1], F32, tag="varg")
                    nc.vector.tensor_sub(out=varg, in0=gsb[:, 1:2], in1=mg2)
                    sd = gn.tile([GT, 1], F32, tag="sd")
                    nc.scalar.activation(
                        out=sd, in_=varg,
                        func=mybir.ActivationFunctionType.Sqrt,
                        bias=eps_sb, scale=1.0,
                    )
                    g2 = gn.tile([GT, 2], F32, tag="g2")
                    nc.vector.tensor_copy(out=g2[:, 0:1], in_=gsb[:, 0:1])
                    nc.vector.reciprocal(out=g2[:, 1:2], in_=sd)

                    bc = gnps.tile([P, 2], F32, tag="bc")
                    nc.tensor.matmul(bc, indb_sb, g2, start=True, stop=True)
                    nc.vector.tensor_mul(out=scale_sb[t], in0=gamma_sb[t], in1=bc[:, 1:2])
                    sh1 = gn.tile([P, 1], F32, tag="sh1")
                    nc.vector.tensor_mul(out=sh1, in0=bc[:, 0:1], in1=scale_sb[t])
                    nc.vector.tensor_sub(out=shift_sb[t], in0=beta_sb[t], in1=sh1)

                # ---- fp8 weights (scaled by 64 * GN scale) + effective biases ----
                wq8 = const.tile([P, CT, D], FP8, tag="wq8")
                wk8 = const.tile([P, CT, D], FP8, tag="wk8")
                wv8 = const.tile([P, CT, C], FP8, tag="wv8")
                wp8 = const.tile([P, CT, C], FP8, tag="wp8")
                s64 = [small.tile([P, 1], F32, tag=f"s64_{t}", name=f"s64_{t}") for t in range(CT)]
                for t in range(CT):
                    nc.vector.tensor_scalar_mul(out=s64[t], in0=scale_sb[t], scalar1=WS)
                    nc.vector.tensor_scalar_mul(out=wq8[:, t, :], in0=wqt_sb[t], scalar1=s64[t])
                    nc.vector.tensor_scalar_mul(out=wk8[:, t, :], in0=wkt_sb[t], scalar1=s64[t])
                    nc.vector.tensor_scalar_mul(out=wv8[:, t, :], in0=wvt_sb[t], scalar1=s64[t])
                    nc.vector.tensor_scalar_mul(out=wp8[:, t, :], in0=wpt_sb[t], scalar1=WS)

                with tc.tile_pool(name="bps", bufs=1, space="PSUM") as bps:
                    bq64 = small.tile([D, 1], F32, tag="bq64")
                    bk64 = small.tile([D, 1], F32, tag="bk64")
                    psq = bps.tile([D, 1], F32, tag="pq")
                    psk = bps.tile([D, 1], F32, tag="pk")
                    for t in range(CT):
                        nc.tensor.matmul(psq, wqt_sb[t], shift_sb[t], start=(t == 0), stop=(t == CT - 1))
                        nc.tensor.matmul(psk, wkt_sb[t], shift_sb[t], start=(t == 0), stop=(t == CT - 1))
                    nc.vector.scalar_tensor_tensor(
                        out=bq64, in0=psq, scalar=1.0, in1=bq_sb, op0=MULT, op1=ADD)
                    nc.vector.tensor_scalar_mul(out=bq64, in0=bq64, scalar1=WS)
                    nc.vector.scalar_tensor_tensor(
                        out=bk64, in0=psk, scalar=1.0, in1=bk_sb, op0=MULT, op1=ADD)
                    nc.vector.tensor_scalar_mul(out=bk64, in0=bk64, scalar1=WS)

                    bv_eff = [small.tile([P, 1], F32, tag=f"bve{e}", name=f"bve{e}") for e in range(CT)]
                    for e in range(CT):
                        ps3 = bps.tile([P, 1], F32, tag=f"pv{e}", name=f"psv{e}")
                        for t in range(CT):
                            nc.tensor.matmul(
                                ps3, wvt_sb[t][:, e * P:(e + 1) * P], shift_sb[t],
                                start=(t == 0), stop=(t == CT - 1),
                            )
                        nc.vector.tensor_add(out=bv_eff[e], in0=ps3, in1=bv_sb[e])
                    for f in range(CT):
                        ps4 = bps.tile([P, 1], F32, tag=f"pp{f}", name=f"psp{f}")
                        for e in range(CT):
                            nc.tensor.matmul(
                                ps4, wpt_sb[e][:, f * P:(f + 1) * P], bv_eff[e],
                                start=(e == 0), stop=(e == CT - 1),
                            )
                        bp_eff = small.tile([P, 1], F32, tag=f"bpe{f}", name=f"bpe{f}")
                        nc.vector.tensor_add(out=bp_eff, in0=ps4, in1=bp_sb[f])
                        nc.vector.tensor_scalar_add(out=xqb[f], in0=xq_f[f], scalar1=bp_eff)

            # ---- q/k projections + v^T pairs 0/1 (rest made inside chunk 0) ----
            q8 = const.tile([D, NQ], FP8, tag="q8")
            k8 = const.tile([D, N], FP8, tag="k8")
            vt8 = const.tile([P, NMB, C], FP8, tag="vt8")

            def make_v_pair(pool, pr):
                vp = pool.tile([P, 2, C], F32, tag="pj", name=f"vp{pr}")
                for mloc in range(2):
                    mb = pr * 2 + mloc
                    ms = slice(mb * MB, (mb + 1) * MB)
                    nc.tensor.matmul(vp[:, mloc, :], x8[:, :, ms], wv8,
                                     start=True, stop=True, perf_mode=DR)
                nc.vector.tensor_copy(out=vt8[:, pr * 2:(pr + 1) * 2, :], in_=vp)

            with tc.tile_pool(name="qkps", bufs=2, space="PSUM") as qkps, \
                 tc.tile_pool(name="vtps", bufs=2, space="PSUM") as vtps:
                def project(dst, bias, w8, ns):
                    pr_ = qkps.tile([D, CH], F32, tag="kp", name="prj")
                    nc.tensor.matmul(pr_, w8, x8[:, :, ns], start=True, stop=True,
                                     perf_mode=DR)
                    nc.scalar.activation(
                        out=dst, in_=pr_,
                        func=mybir.ActivationFunctionType.Identity,
                        bias=bias, scale=1.0,
                    )

                project(q8[:, 0:CH], bq64, wq8, slice(0, CH))
                for ck in range(N // CH):
                    ns = slice(ck * CH, (ck + 1) * CH)
                    project(k8[:, ns], bk64, wk8, ns)
                for cq in range(1, NQ // CH):
                    ns = slice(cq * CH, (cq + 1) * CH)
                    project(q8[:, ns], bq64, wq8, ns)
                make_v_pair(vtps, 0)
                make_v_pair(vtps, 1)

            # ---- attention ----
            ones_b = ones8[:, None, :].broadcast_to([P, CT, MB])
            with tc.tile_pool(name="stps", bufs=2, space="PSUM") as stps, \
                 tc.tile_pool(name="attps", bufs=1, space="PSUM") as attps, \
                 tc.tile_pool(name="rsps", bufs=1, space="PSUM") as rsps, \
                 tc.tile_pool(name="pjps", bufs=1, space="PSUM") as pjps, \
                 tc.tile_pool(name="pp", bufs=6) as pp, \
                 tc.tile_pool(name="attsb", bufs=2) as attsb, \
                 tc.tile_pool(name="osb", bufs=4) as osb, \
                 tc.tile_pool(name="rsb", bufs=2) as rsb:
                pend = None

                def emit_epilogue(ep):
                    ns_p, att2_p, rs_p = ep
                    rec = rsb.tile([P, CH], F32, tag="rec")
                    nc.vector.reciprocal_approx_fast(out=rec, in_=rs_p)
                    att8 = attsb.tile([P, CT, CH], FP8, tag="att8")
                    for i in range(CT):
                        nc.vector.tensor_mul(out=att8[:, i, :], in0=att2_p[:, i, :], in1=rec)
                    for f in range(CT):
                        fs = slice(f * P, (f + 1) * P)
                        pj = pjps.tile([P, CH], F32, tag="pj", name=f"pj{f}")
                        nc.tensor.matmul(pj, wp8[:, :, fs], att8,
                                         start=True, stop=True, perf_mode=DR)
                        o = osb.tile([P, CH], F32, tag="o")
                        nc.vector.scalar_tensor_tensor(
                            out=o, in0=pj, scalar=1.0 / (WS * WS),
                            in1=xqb[f][:, ns_p], op0=MULT, op1=ADD)
                        nc.sync.dma_start(out=out_ext[fs, ns_p], in_=o)

                for ch in range(NCH):
                    ns = slice(ch * CH, (ch + 1) * CH)
                    att2 = attps.tile([P, CT, CH], F32, tag="att2")
                    rs = rsps.tile([P, CH], F32, tag="rs")
                    q_b = q8[:, None, ns].broadcast_to([D, CT, CH])

                    p_tiles = [None] * NPR
                    for g in range(NPR + 1):
                        if g < NPR:
                            stg = stps.tile([P, CT, CH], F32, tag="stg")
                            for j in range(2):
                                mb = g * 2 + j
                                k_b = k8[:, None, mb * MB:(mb + 1) * MB].broadcast_to([D, CT, MB])
                                nc.tensor.matmul(stg[:, j, :], k_b, q_b,
                                                 start=True, stop=True, perf_mode=DR)
                            pg = pp.tile([P, CT, CH], FP8, tag="pg")
                            nc.scalar.activation(
                                out=pg, in_=stg,
                                func=mybir.ActivationFunctionType.Exp,
                                scale=EXP_SCALE,
                            )
                            p_tiles[g] = pg
                        if g == 1 and pend is not None:
                            emit_epilogue(pend)
                            pend = None
                        if g >= 1:
                            gp = g - 1
                            pg = p_tiles[gp]
                            nc.tensor.matmul(rs, ones_b, pg,
                                             start=(gp == 0), stop=(gp == NPR - 1),
                                             perf_mode=DR)
                            for e in range(CT):
                                nc.tensor.matmul(
                                    att2[:, e, :],
                                    vt8[:, 2 * gp:2 * gp + 2, e * P:(e + 1) * P],
                                    pg,
                                    start=(gp == 0), stop=(gp == NPR - 1),
                                    perf_mode=DR,
                                )
                            # chunk 0 doubles as the v^T production phase:
                            # pair gp+2 is built in the (otherwise idle) pj bank
                            if ch == 0 and gp + 2 < NPR:
                                make_v_pair(pjps, gp + 2)
                    pend = (ns, att2, rs)
                emit_epilogue(pend)

    nc.compile()
    _CACHE["nc"] = nc
    return nc


def _make_in_maps(x, gamma, beta, wq, bq, wk, bk, wv, bv, wp, bp):
    x = np.ascontiguousarray(np.asarray(x, dtype=np.float32))

    GT = G // CT
    ind16 = np.zeros((P, GT), np.float32)
    for c in range(P):
        ind16[c, c // GS] = 1.0 / GS
    indb = np.zeros((GT, P), np.float32)
    for c in range(P):
        indb[c // GS, c] = 1.0

    wall = np.concatenate(
        [
            np.asarray(wq, np.float32).T,
            np.asarray(wk, np.float32).T,
            np.asarray(wv, np.float32).T,
            np.asarray(wp, np.float32).T,
        ],
        axis=1,
    )
    bc4 = np.stack(
        [
            np.asarray(gamma, np.float32),
            np.asarray(beta, np.float32),
            np.asarray(bv, np.float32),
            np.asarray(bp, np.float32),
        ],
        axis=1,
    )
    bqk = np.stack([np.asarray(bq, np.float32), np.asarray(bk, np.float32)], axis=1)

    common = {
        "wall": np.ascontiguousarray(wall),
        "bc4": np.ascontiguousarray(bc4),
        "bqk": np.ascontiguousarray(bqk),
        "ind16": ind16,
        "indb": indb,
    }

    xf = x.reshape(B, C, N)
    # x8[p, t*N + n] = x[t*128+p, n] in fp8
    x8_all = np.ascontiguousarray(
        xf.reshape(B, CT, P, N).transpose(0, 2, 1, 3).reshape(B, P, CT * N)
    ).astype(ml_dtypes.float8_e4m3)
    in_maps = []
    for core in range(8):
        b, half = core // 2, core % 2
        m = dict(common)
        m["x8"] = x8_all[b]
        m["xq"] = np.ascontiguousarray(xf[b][:, half * NQ:(half + 1) * NQ])
        in_maps.append(m)
    return in_maps


def kernel(x, gamma, beta, wq, bq, wk, bk, wv, bv, wp, bp):
    nc = _build()
    in_maps = _make_in_maps(x, gamma, beta, wq, bq, wk, bk, wv, bv, wp, bp)
    global _last_in_maps
    _last_in_maps = in_maps
    res = run_bass_kernel_spmd(nc, in_maps, list(range(8)))

    y = np.empty((B, C, N), np.float32)
    for core in range(8):
        b, half = core // 2, core % 2
        y[b][:, half * NQ:(half + 1) * NQ] = res.results[core]["out"]
    return y.reshape(B, C, H, W)


# revision 5
# speedup vs baseline: 1.0723x; 1.0621x over previous
"""AttentionBlock (GroupNorm + 1x1-conv QKV self-attention + residual) on 8 TRN2 cores.

Sharding: data-parallel over batch B=4 x sequence-parallel over the 4096
tokens (2 cores per batch element, each handling 2048 query rows; K/V and
GroupNorm are computed redundantly per core pair — they are cheap relative
to attention).

Per-core device kernel — all heavy matmuls in fp8e4 DoubleRow perf mode
(contraction-256 ops run in a single PE pass):
  - x ships as fp8 [128, 2, N] (the 2 = channel k-tiles) + the core's query
    half in fp32 (exact residual). GroupNorm stats are split across engines:
    channel tile 0 via DVE bn_stats, tile 1 via scalar-engine Copy/Square
    activations with accum_out. GN is folded into the QKV weights, which
    are scaled by 64 into fp8's healthy range.
  - q/k/v projections are single DoubleRow matmuls (contraction C=256 via
    k-tiles). v^T pairs 2..15 are produced inside chunk 0 of the attention
    loop (the projection PSUM bank is idle there), so the softmax pipeline
    starts ~10us earlier.
  - S^T is a plain fp8 K=32 matmul (the PE is power-throttled when array
    utilization is high, so avoiding redundant MACs matters more than
    packing). exp on the scalar engine, output fp8.
  - The softmax denominator is an M=1 ones DoubleRow matmul accumulated
    over key pairs (one PE column active - minimal array energy), then
    reciprocal_approx_fast + gpsimd partition_broadcast.
  - P*V accumulates key-block pairs per DoubleRow matmul into att2; the
    epilogue divides by the denominator during the fp8 evacuation
    (att8 = att2 * rec = 64*att), projects with fp8 DoubleRow, and fuses
    residual-add + 1/4096 rescale in one scalar_tensor_tensor.
"""
import sys

sys.path.insert(0, "/opt/trn_rl_repo")

import ml_dtypes
import numpy as np

import concourse.bass as bass
import concourse.tile as tile
from concourse import bacc, mybir
from concourse.bass_utils import run_bass_kernel_spmd

F32 = mybir.dt.float32
BF16 = mybir.dt.bfloat16
FP8 = mybir.dt.float8e4
DR = mybir.MatmulPerfMode.DoubleRow
MULT = mybir.AluOpType.mult
ADD = mybir.AluOpType.add

B, C, H, W = 4, 256, 64, 64
N = H * W          # 4096 tokens
NQ = N // 2        # 2048 query rows per core
D = C // 8         # 32 qk dim
G = 32             # groups
GS = C // G        # 8 channels per group
EPS = 1e-5
P = 128            # partitions
CT = C // P        # 2 channel tiles
CH = 512           # nq chunk
NCH = NQ // CH     # 4 chunks
MB = 128           # m block
NMB = N // MB      # 32 m blocks
NPR = NMB // 2     # 16 m-block pairs
WS = 64.0          # fp8 weight scale
SM_SCALE = float(D) ** -0.5
EXP_SCALE = SM_SCALE / (WS * WS)
NWARM = 32         # PE p-state warmup matmuls

_CACHE = {}
_last_in_maps = None


def _build():
    if "nc" in _CACHE:
        return _CACHE["nc"]

    nc = bacc.Bacc("TRN2", target_bir_lowering=False, debug=False, num_devices=8)

    WALL = 2 * D + 2 * C  # wq|wk|wv|wp columns, transposed, concatenated
    x8_ext = nc.declare_dram_parameter("x8", [P, CT * N], FP8, isOutput=False)
    xq_ext = nc.declare_dram_parameter("xq", [C, NQ], F32, isOutput=False)
    wall_ext = nc.declare_dram_parameter("wall", [C, WALL], F32, isOutput=False)
    bc4_ext = nc.declare_dram_parameter("bc4", [C, 4], F32, isOutput=False)  # gamma|beta|bv|bp
    bqk_ext = nc.declare_dram_parameter("bqk", [D, 2], F32, isOutput=False)  # bq|bk
    ind16_ext = nc.declare_dram_parameter("ind16", [P, G // CT], F32, isOutput=False)
    indb_ext = nc.declare_dram_parameter("indb", [G // CT, P], F32, isOutput=False)
    out_ext = nc.declare_dram_parameter("out", [C, NQ], F32, isOutput=True)

    GT = G // CT  # 16 groups per channel tile
    NHALF = N // 2

    with tile.TileContext(nc) as tc:
        with tc.tile_pool(name="const", bufs=1) as const, \
             tc.tile_pool(name="small", bufs=1) as small:
            ones8 = small.tile([P, 1], FP8, tag="ones8")
            nc.vector.memset(ones8, 1.0)

            # ---- input DMAs: x8 quadrants first, then weights, then xq ----
            x8 = const.tile([P, CT, N], FP8, tag="x8", name="x8")
            nc.sync.dma_start(out=x8[:, 0, 0:NHALF], in_=x8_ext[:, 0:NHALF])
            nc.scalar.dma_start(out=x8[:, 0, NHALF:N], in_=x8_ext[:, NHALF:N])
            nc.gpsimd.dma_start(out=x8[:, 1, 0:NHALF], in_=x8_ext[:, N:N + NHALF])
            nc.sync.dma_start(out=x8[:, 1, NHALF:N], in_=x8_ext[:, N + NHALF:2 * N])

            wall_sb = []
            for t in range(CT):
                cs = slice(t * P, (t + 1) * P)
                wl = const.tile([P, WALL], F32, tag=f"wall{t}", name=f"wall{t}")
                (nc.gpsimd if t == 0 else nc.scalar).dma_start(out=wl, in_=wall_ext[cs, :])
                wall_sb.append(wl)
            wqt_sb = [wall_sb[t][:, 0:D] for t in range(CT)]
            wkt_sb = [wall_sb[t][:, D:2 * D] for t in range(CT)]
            wvt_sb = [wall_sb[t][:, 2 * D:2 * D + C] for t in range(CT)]
            wpt_sb = [wall_sb[t][:, 2 * D + C:WALL] for t in range(CT)]

            bc4_sb = []
            for t in range(CT):
                cs = slice(t * P, (t + 1) * P)
                b4 = small.tile([P, 4], F32, tag=f"bc4{t}", name=f"bc4{t}")
                nc.sync.dma_start(out=b4, in_=bc4_ext[cs, :])
                bc4_sb.append(b4)
            gamma_sb = [bc4_sb[t][:, 0:1] for t in range(CT)]
            beta_sb = [bc4_sb[t][:, 1:2] for t in range(CT)]
            bv_sb = [bc4_sb[t][:, 2:3] for t in range(CT)]
            bp_sb = [bc4_sb[t][:, 3:4] for t in range(CT)]
            bqk_sb = small.tile([D, 2], F32, tag="bqk")
            nc.sync.dma_start(out=bqk_sb, in_=bqk_ext[:])
            bq_sb, bk_sb = bqk_sb[:, 0:1], bqk_sb[:, 1:2]
            ind16_sb = small.tile([P, GT], F32, tag="ind16")
            nc.sync.dma_start(out=ind16_sb, in_=ind16_ext[:])
            indb_sb = small.tile([GT, P], F32, tag="indb")
            nc.sync.dma_start(out=indb_sb, in_=indb_ext[:])
            eps_sb = small.tile([GT, 1], F32, tag="eps")
            nc.vector.memset(eps_sb, EPS)

            xq_f = []
            for t in range(CT):
                cs = slice(t * P, (t + 1) * P)
                xqt = const.tile([P, NQ], F32, tag=f"xqt{t}", name=f"xqt{t}")
                (nc.gpsimd if t == 0 else nc.scalar).dma_start(out=xqt, in_=xq_ext[cs, :])
                xq_f.append(xqt)

            xqb = [const.tile([P, NQ], F32, tag=f"xqb{t}", name=f"xqb{t}") for t in range(CT)]
            scale_sb = [small.tile([P, 1], F32, tag=f"scale{t}", name=f"scale{t}") for t in range(CT)]
            shift_sb = [small.tile([P, 1], F32, tag=f"shift{t}", name=f"shift{t}") for t in range(CT)]

            # ---- GroupNorm stats: tile 0 on DVE (bn_stats), tile 1 on the
            # scalar engine (Copy/Square + accum_out), in DMA-half granules ----
            with tc.tile_pool(name="gn", bufs=2) as gn, \
                 tc.tile_pool(name="gnps", bufs=1, space="PSUM") as gnps:
                mx_t = []
                # tile 0: DVE
                stats = gn.tile([P, 8, nc.vector.BN_STATS_DIM], F32, tag="st")
                for s in range(N // 512):
                    nc.vector.bn_stats(out=stats[:, s, :], in_=x8[:, 0, s * 512:(s + 1) * 512])
                mv = gn.tile([P, nc.vector.BN_AGGR_DIM], F32, tag="mv")
                nc.vector.bn_aggr(out=mv, in_=stats)
                mx0 = gn.tile([P, 2], F32, tag="mx0")
                nc.vector.tensor_copy(out=mx0[:, 0:1], in_=mv[:, 0:1])
                msq = gn.tile([P, 1], F32, tag="msq")
                nc.vector.tensor_mul(out=msq, in0=mv[:, 0:1], in1=mv[:, 0:1])
                nc.vector.tensor_add(out=mx0[:, 1:2], in0=mv[:, 1:2], in1=msq)
                mx_t.append(mx0)
                # tile 1: scalar engine accumulators
                scr = gn.tile([P, NHALF], FP8, tag="scr")
                acc = gn.tile([P, 4], F32, tag="acc")  # sum_h0|sum_h1|sq_h0|sq_h1
                for hh in range(2):
                    xs = x8[:, 1, hh * NHALF:(hh + 1) * NHALF]
                    nc.scalar.activation(
                        out=scr, in_=xs, func=mybir.ActivationFunctionType.Copy,
                        accum_out=acc[:, hh:hh + 1])
                for hh in range(2):
                    xs = x8[:, 1, hh * NHALF:(hh + 1) * NHALF]
                    nc.scalar.activation(
                        out=scr, in_=xs, func=mybir.ActivationFunctionType.Square,
                        accum_out=acc[:, 2 + hh:3 + hh])
                mx1 = gn.tile([P, 2], F32, tag="mx1")
                ssum = gn.tile([P, 2], F32, tag="ssum")
                nc.vector.tensor_add(out=ssum[:, 0:1], in0=acc[:, 0:1], in1=acc[:, 1:2])
                nc.vector.tensor_add(out=ssum[:, 1:2], in0=acc[:, 2:3], in1=acc[:, 3:4])
                nc.vector.tensor_scalar_mul(out=mx1, in0=ssum, scalar1=1.0 / float(N))
                mx_t.append(mx1)

                for t in range(CT):
                    gps = gnps.tile([GT, 2], F32, tag="gps")
                    nc.tensor.matmul(gps, ind16_sb, mx_t[t], start=True, stop=True)
                    gsb = gn.tile([GT, 2], F32, tag="gsb")
                    nc.vector.tensor_copy(out=gsb, in_=gps)
                    mg2 = gn.tile([GT, 1], F32, tag="mg2")
                    nc.vector.tensor_mul(out=mg2, in0=gsb[:, 0:1], in1=gsb[:, 0:1])
                    varg = gn.tile([GT, 1], F32, tag="varg")
                    nc.vector.tensor_sub(out=varg, in0=gsb[:, 1:2], in1=mg2)
                    sd = gn.tile([GT, 1], F32, tag="sd")
                    nc.scalar.activation(
                        out=sd, in_=varg,
                        func=mybir.ActivationFunctionType.Sqrt,
                        bias=eps_sb, scale=1.0,
                    )
                    g2 = gn.tile([GT, 2], F32, tag="g2")
                    nc.vector.tensor_copy(out=g2[:, 0:1], in_=gsb[:, 0:1])
                    nc.vector.reciprocal(out=g2[:, 1:2], in_=sd)

                    bc = gnps.tile([P, 2], F32, tag="bc")
                    nc.tensor.matmul(bc, indb_sb, g2, start=True, stop=True)
                    nc.vector.tensor_mul(out=scale_sb[t], in0=gamma_sb[t], in1=bc[:, 1:2])
                    sh1 = gn.tile([P, 1], F32, tag="sh1")
                    nc.vector.tensor_mul(out=sh1, in0=bc[:, 0:1], in1=scale_sb[t])
                    nc.vector.tensor_sub(out=shift_sb[t], in0=beta_sb[t], in1=sh1)

                # ---- fp8 weights (scaled by 64 * GN scale) + effective biases ----
                wq8 = const.tile([P, CT, D], FP8, tag="wq8")
                wk8 = const.tile([P, CT, D], FP8, tag="wk8")
                wv8 = const.tile([P, CT, C], FP8, tag="wv8")
                wp8 = const.tile([P, CT, C], FP8, tag="wp8")
                s64 = [small.tile([P, 1], F32, tag=f"s64_{t}", name=f"s64_{t}") for t in range(CT)]
                for t in range(CT):
                    nc.vector.tensor_scalar_mul(out=s64[t], in0=scale_sb[t], scalar1=WS)
                    nc.vector.tensor_scalar_mul(out=wq8[:, t, :], in0=wqt_sb[t], scalar1=s64[t])
                    nc.vector.tensor_scalar_mul(out=wk8[:, t, :], in0=wkt_sb[t], scalar1=s64[t])
                    nc.vector.tensor_scalar_mul(out=wv8[:, t, :], in0=wvt_sb[t], scalar1=s64[t])
                    nc.vector.tensor_scalar_mul(out=wp8[:, t, :], in0=wpt_sb[t], scalar1=WS)

                with tc.tile_pool(name="bps", bufs=1, space="PSUM") as bps:
                    bq64 = small.tile([D, 1], F32, tag="bq64")
                    bk64 = small.tile([D, 1], F32, tag="bk64")
                    psq = bps.tile([D, 1], F32, tag="pq")
                    psk = bps.tile([D, 1], F32, tag="pk")
                    for t in range(CT):
                        nc.tensor.matmul(psq, wqt_sb[t], shift_sb[t], start=(t == 0), stop=(t == CT - 1))
                        nc.tensor.matmul(psk, wkt_sb[t], shift_sb[t], start=(t == 0), stop=(t == CT - 1))
                    nc.vector.scalar_tensor_tensor(
                        out=bq64, in0=psq, scalar=1.0, in1=bq_sb, op0=MULT, op1=ADD)
                    nc.vector.tensor_scalar_mul(out=bq64, in0=bq64, scalar1=WS)
                    nc.vector.scalar_tensor_tensor(
                        out=bk64, in0=psk, scalar=1.0, in1=bk_sb, op0=MULT, op1=ADD)
                    nc.vector.tensor_scalar_mul(out=bk64, in0=bk64, scalar1=WS)

                    bv_eff = [small.tile([P, 1], F32, tag=f"bve{e}", name=f"bve{e}") for e in range(CT)]
                    for e in range(CT):
                        ps3 = bps.tile([P, 1], F32, tag=f"pv{e}", name=f"psv{e}")
                        for t in range(CT):
                            nc.tensor.matmul(
                                ps3, wvt_sb[t][:, e * P:(e + 1) * P], shift_sb[t],
                                start=(t == 0), stop=(t == CT - 1),
                            )
                        nc.vector.tensor_add(out=bv_eff[e], in0=ps3, in1=bv_sb[e])
                    for f in range(CT):
                        ps4 = bps.tile([P, 1], F32, tag=f"pp{f}", name=f"psp{f}")
                        for e in range(CT):
                            nc.tensor.matmul(
                                ps4, wpt_sb[e][:, f * P:(f + 1) * P], bv_eff[e],
                                start=(e == 0), stop=(e == CT - 1),
                            )
                        bp_eff = small.tile([P, 1], F32, tag=f"bpe{f}", name=f"bpe{f}")
                        nc.vector.tensor_add(out=bp_eff, in0=ps4, in1=bp_sb[f])
                        nc.vector.tensor_scalar_add(out=xqb[f], in0=xq_f[f], scalar1=bp_eff)

            # ---- q/k projections + v^T pairs 0/1 (rest made inside chunk 0) ----
            q8 = const.tile([D, NQ], FP8, tag="q8")
            k8 = const.tile([D, N], FP8, tag="k8")
            vt8 = const.tile([P, NMB, C], FP8, tag="vt8")

            def make_v_pair(pool, pr):
                vp = pool.tile([P, 2, C], F32, tag="pj", name=f"vp{pr}")
                for mloc in range(2):
                    mb = pr * 2 + mloc
                    ms = slice(mb * MB, (mb + 1) * MB)
                    nc.tensor.matmul(vp[:, mloc, :], x8[:, :, ms], wv8,
                                     start=True, stop=True, perf_mode=DR)
                nc.vector.tensor_copy(out=vt8[:, pr * 2:(pr + 1) * 2, :], in_=vp)

            with tc.tile_pool(name="qkps", bufs=2, space="PSUM") as qkps, \
                 tc.tile_pool(name="vtps", bufs=2, space="PSUM") as vtps:
                def project(dst, bias, w8, ns):
                    pr_ = qkps.tile([D, CH], F32, tag="kp", name="prj")
                    nc.tensor.matmul(pr_, w8, x8[:, :, ns], start=True, stop=True,
                                     perf_mode=DR)
                    nc.scalar.activation(
                        out=dst, in_=pr_,
                        func=mybir.ActivationFunctionType.Identity,
                        bias=bias, scale=1.0,
                    )

                project(q8[:, 0:CH], bq64, wq8, slice(0, CH))
                for ck in range(N // CH):
                    ns = slice(ck * CH, (ck + 1) * CH)
                    project(k8[:, ns], bk64, wk8, ns)
                for cq in range(1, NQ // CH):
                    ns = slice(cq * CH, (cq + 1) * CH)
                    project(q8[:, ns], bq64, wq8, ns)
                make_v_pair(vtps, 0)
                make_v_pair(vtps, 1)

            # ---- attention ----
            ones_b = ones8[:, None, :].broadcast_to([P, CT, 1])
            with tc.tile_pool(name="stps", bufs=2, space="PSUM") as stps, \
                 tc.tile_pool(name="attps", bufs=1, space="PSUM") as attps, \
                 tc.tile_pool(name="rsps", bufs=1, space="PSUM") as rsps, \
                 tc.tile_pool(name="pjps", bufs=1, space="PSUM") as pjps, \
                 tc.tile_pool(name="pp", bufs=6) as pp, \
                 tc.tile_pool(name="attsb", bufs=2) as attsb, \
                 tc.tile_pool(name="osb", bufs=4) as osb, \
                 tc.tile_pool(name="rsb", bufs=2) as rsb:
                pend = None

                def emit_epilogue(ep):
                    ns_p, att2_p, rs_p = ep
                    rec1 = rsb.tile([1, CH], F32, tag="rec1")
                    nc.vector.reciprocal_approx_fast(out=rec1, in_=rs_p)
                    rec = rsb.tile([P, CH], F32, tag="rec")
                    nc.gpsimd.partition_broadcast(rec, rec1, channels=P)
                    att8 = attsb.tile([P, CT, CH], FP8, tag="att8")
                    for i in range(CT):
                        nc.vector.tensor_mul(out=att8[:, i, :], in0=att2_p[:, i, :], in1=rec)
                    for f in range(CT):
                        fs = slice(f * P, (f + 1) * P)
                        pj = pjps.tile([P, CH], F32, tag="pj", name=f"pj{f}")
                        nc.tensor.matmul(pj, wp8[:, :, fs], att8,
                                         start=True, stop=True, perf_mode=DR)
                        o = osb.tile([P, CH], F32, tag="o")
                        nc.vector.scalar_tensor_tensor(
                            out=o, in0=pj, scalar=1.0 / (WS * WS),
                            in1=xqb[f][:, ns_p], op0=MULT, op1=ADD)
                        nc.sync.dma_start(out=out_ext[fs, ns_p], in_=o)

                for ch in range(NCH):
                    ns = slice(ch * CH, (ch + 1) * CH)
                    att2 = attps.tile([P, CT, CH], F32, tag="att2")
                    rs = rsps.tile([1, CH], F32, tag="rs")
                    p_tiles = [None] * NPR
                    for g in range(NPR + 1):
                        if g < NPR:
                            stg = stps.tile([P, CT, CH], F32, tag="stg")
                            for j in range(2):
                                mb = g * 2 + j
                                nc.tensor.matmul(stg[:, j, :],
                                                 k8[:, mb * MB:(mb + 1) * MB],
                                                 q8[:, ns],
                                                 start=True, stop=True)
                            pg = pp.tile([P, CT, CH], FP8, tag="pg")
                            nc.scalar.activation(
                                out=pg, in_=stg,
                                func=mybir.ActivationFunctionType.Exp,
                                scale=EXP_SCALE,
                            )
                            p_tiles[g] = pg
                        if g == 1 and pend is not None:
                            emit_epilogue(pend)
                            pend = None
                        if g >= 1:
                            gp = g - 1
                            pg = p_tiles[gp]
                            nc.tensor.matmul(rs, ones_b, pg,
                                             start=(gp == 0), stop=(gp == NPR - 1),
                                             perf_mode=DR)
                            for e in range(CT):
                                nc.tensor.matmul(
                                    att2[:, e, :],
                                    vt8[:, 2 * gp:2 * gp + 2, e * P:(e + 1) * P],
                                    pg,
                                    start=(gp == 0), stop=(gp == NPR - 1),
                                    perf_mode=DR,
                                )
                            # chunk 0 doubles as the v^T production phase:
                            # pair gp+2 is built in the (otherwise idle) pj bank
                            if ch == 0 and gp + 2 < NPR:
                                make_v_pair(pjps, gp + 2)
                    pend = (ns, att2, rs)
                emit_epilogue(pend)

    nc.compile()
    _CACHE["nc"] = nc
    return nc


def _make_in_maps(x, gamma, beta, wq, bq, wk, bk, wv, bv, wp, bp):
    x = np.ascontiguousarray(np.asarray(x, dtype=np.float32))

    GT = G // CT
    ind16 = np.zeros((P, GT), np.float32)
    for c in range(P):
        ind16[c, c // GS] = 1.0 / GS
    indb = np.zeros((GT, P), np.float32)
    for c in range(P):
        indb[c // GS, c] = 1.0

    wall = np.concatenate(
        [
            np.asarray(wq, np.float32).T,
            np.asarray(wk, np.float32).T,
            np.asarray(wv, np.float32).T,
            np.asarray(wp, np.float32).T,
        ],
        axis=1,
    )
    bc4 = np.stack(
        [
            np.asarray(gamma, np.float32),
            np.asarray(beta, np.float32),
            np.asarray(bv, np.float32),
            np.asarray(bp, np.float32),
        ],
        axis=1,
    )
    bqk = np.stack([np.asarray(bq, np.float32), np.asarray(bk, np.float32)], axis=1)

    common = {
        "wall": np.ascontiguousarray(wall),
        "bc4": np.ascontiguousarray(bc4),
        "bqk": np.ascontiguousarray(bqk),
        "ind16": ind16,
        "indb": indb,
    }

    xf = x.reshape(B, C, N)
    # x8[p, t*N + n] = x[t*128+p, n] in fp8
    x8_all = np.ascontiguousarray(
        xf.reshape(B, CT, P, N).transpose(0, 2, 1, 3).reshape(B, P, CT * N)
    ).astype(ml_dtypes.float8_e4m3)
    in_maps = []
    for core in range(8):
        b, half = core // 2, core % 2
        m = dict(common)
        m["x8"] = x8_all[b]
        m["xq"] = np.ascontiguousarray(xf[b][:, half * NQ:(half + 1) * NQ])
        in_maps.append(m)
    return in_maps


def kernel(x, gamma, beta, wq, bq, wk, bk, wv, bv, wp, bp):
    nc = _build()
    in_maps = _make_in_maps(x, gamma, beta, wq, bq, wk, bk, wv, bv, wp, bp)
    global _last_in_maps
    _last_in_maps = in_maps
    res = run_bass_kernel_spmd(nc, in_maps, list(range(8)))

    y = np.empty((B, C, N), np.float32)
    for core in range(8):
        b, half = core // 2, core % 2
        y[b][:, half * NQ:(half + 1) * NQ] = res.results[core]["out"]
    return y.reshape(B, C, H, W)
